# revision 2
# baseline (speedup 1.0000x reference)
"""Trainium2 Bass kernel for nn_Attn (Bahdanau-style attention scores).

Computation (per batch b of B=128):
    energy = tanh(enc[b] @ We.T + (hidden @ Wh.T)[b] + bias)   # (L, H)
    scores = energy @ v                                        # (L,)
    out[b] = softmax(scores)                                   # (1, L)

Sharding: batch data-parallel over 8 NeuronCores (16 batches/core);
weights replicated. Per core the dominant matmul is computed in the
[h, l] orientation so the PE tensor engine contracts over d (=576):

    part_e[h, l] = sum_d WeT[d, h] * encT[d, l]      (lhsT=WeT, rhs=encT)

which lets the (hidden@Wh.T + bias) term fuse into the tanh as a
per-partition activation bias, and the v-contraction run as a second
PE matmul (lhsT = v column, contracting over h on partitions).
Matmuls run as float32r (full fp32 data, reduced-precision multiply,
1 col/cycle on TRN2 vs 4 for exact fp32).

Host side: encoder_outputs (L, B, D) is transposed once to (B, D, L) so
each per-batch d-major tile DMA is contiguous.
"""

import numpy as np

import concourse.bacc as bacc
import concourse.bass as bass
import concourse.mybir as mybir
import concourse.tile as tile
from concourse import bass_utils
from concourse.mybir import ActivationFunctionType as AF
from concourse.mybir import AluOpType, AxisListType

N_CORES = 8
B, L, H = 128, 1024, 512
ONEHOT = 64
DE = H + ONEHOT          # 576, contraction dim of the big matmul
BL = B // N_CORES        # 16 batches per core
F32 = mybir.dt.float32
F32R = mybir.dt.float32r

KT = [128, 128, 128, 128, 64]            # d-dim tiles (sum = 576)
KOFF = [0, 128, 256, 384, 512]
NHT = H // 128                           # 4 h-tiles
NLH = L // 512                           # 2 l-halves (N=512 per matmul)


def build(reps: int = 1):
    """Build + trace the per-core Bass program. Returns the compiled nc."""
    nc = bacc.Bacc(
        "TRN2", target_bir_lowering=False, debug=False, num_devices=N_CORES
    )
    enc = nc.dram_tensor("enc", [BL, DE, L], F32R, kind="ExternalInput").ap()
    hid = nc.dram_tensor("hid", [H, BL], F32, kind="ExternalInput").ap()
    wet = nc.dram_tensor("wet", [DE, H], F32R, kind="ExternalInput").ap()
    wht = nc.dram_tensor("wht", [H, H], F32, kind="ExternalInput").ap()
    bcol = nc.dram_tensor("bcol", [128, NHT], F32, kind="ExternalInput").ap()
    vcol = nc.dram_tensor("vcol", [128, NHT], F32R, kind="ExternalInput").ap()
    out = nc.dram_tensor("out", [BL, L], F32, kind="ExternalOutput").ap()

    with tile.TileContext(nc) as tc:
        with (
            tc.tile_pool(name="const", bufs=1) as cpool,
            tc.tile_pool(name="encp", bufs=3) as epool,
            tc.tile_pool(name="energy", bufs=3) as gpool,
            tc.tile_pool(name="cb", bufs=2) as cbpool,
            tc.tile_pool(name="soft", bufs=4) as spool,
            tc.tile_pool(name="ps1", bufs=4, space="PSUM") as ps1,
            tc.tile_pool(name="ps2", bufs=4, space="PSUM") as ps2,
        ):
            # ---- replicated constants ----
            wet_sb = []
            for kt in range(5):
                t = cpool.tile([KT[kt], H], F32R, tag=f"wet{kt}", name=f"wet{kt}")
                nc.sync.dma_start(t[:], wet[KOFF[kt] : KOFF[kt] + KT[kt], :])
                wet_sb.append(t)
            wht_sb = []
            for kt in range(4):
                t = cpool.tile([128, H], F32, tag=f"wht{kt}", name=f"wht{kt}")
                nc.sync.dma_start(t[:], wht[kt * 128 : (kt + 1) * 128, :])
                wht_sb.append(t)
            hid_sb = []
            for kt in range(4):
                t = cpool.tile([128, BL], F32, tag=f"hid{kt}", name=f"hid{kt}")
                nc.sync.dma_start(t[:], hid[kt * 128 : (kt + 1) * 128, :])
                hid_sb.append(t)
            bcol_sb = cpool.tile([128, NHT], F32, tag="bcol", name="bcol_sb")
            nc.sync.dma_start(bcol_sb[:], bcol[:, :])
            vcol_sb = cpool.tile([128, NHT], F32R, tag="vcol", name="vcol_sb")
            nc.sync.dma_start(vcol_sb[:], vcol[:, :])

            for _rep in range(reps):
                # ---- c[h, b] = (hidden @ Wh.T).T + bias, per-partition h ----
                cb_sb = []
                for ht in range(4):
                    pc = ps1.tile([128, 512], F32, tag="ps1", name=f"pc{ht}")
                    for kt in range(4):
                        nc.tensor.matmul(
                            pc[:, :BL],
                            lhsT=wht_sb[kt][:, ht * 128 : (ht + 1) * 128],
                            rhs=hid_sb[kt][:],
                            start=(kt == 0),
                            stop=(kt == 3),
                        )
                    cbt = cbpool.tile([128, BL], F32, tag=f"cb{ht}", name=f"cb{ht}")
                    nc.vector.tensor_scalar_add(
                        cbt[:], pc[:, :BL], bcol_sb[:, ht : ht + 1]
                    )
                    cb_sb.append(cbt)

                # ---- main loop over local batches ----
                for b in range(BL):
                    et = []
                    for kt in range(5):
                        t = epool.tile([KT[kt], L], F32R, tag=f"enc{kt}", name=f"enc{kt}_{b}")
                        nc.sync.dma_start(
                            t[:], enc[b, KOFF[kt] : KOFF[kt] + KT[kt], :]
                        )
                        et.append(t)

                    ex = spool.tile([1, L], F32, tag="ex", name=f"ex{b}")
                    sums = spool.tile([1, NLH], F32, tag="sums", name=f"sums{b}")
                    for lh in range(NLH):
                        ps_s = ps2.tile([1, 512], F32, tag="ps2", name=f"ps_s{b}_{lh}")
                        for ht in range(4):
                            pe_t = ps1.tile(
                                [128, 512], F32, tag="ps1", name=f"pe{b}_{lh}_{ht}"
                            )
                            for kt in range(5):
                                nc.tensor.matmul(
                                    pe_t[:],
                                    lhsT=wet_sb[kt][
                                        :, ht * 128 : (ht + 1) * 128
                                    ],
                                    rhs=et[kt][
                                        :, lh * 512 : (lh + 1) * 512
                                    ],
                                    start=(kt == 0),
                                    stop=(kt == 4),
                                )
                            en_t = gpool.tile(
                                [128, 512], F32R, tag="en", name=f"en{b}_{lh}_{ht}"
                            )
                            nc.scalar.activation(
                                en_t[:], pe_t[:], AF.Tanh,
                                bias=cb_sb[ht][:, b : b + 1],
                            )
                            nc.tensor.matmul(
                                ps_s[:],
                                lhsT=vcol_sb[:, ht : ht + 1],
                                rhs=en_t[:],
                                start=(ht == 0),
                                stop=(ht == 3),
                            )
                        # exp of this half + running sum (no max subtraction:
                        # |scores| <= ||v||_1 ~ 18, exp() is safe in fp32)
                        nc.scalar.activation(
                            ex[:, lh * 512 : (lh + 1) * 512],
                            ps_s[:],
                            AF.Exp,
                            accum_out=sums[:, lh : lh + 1],
                        )
                    stot = spool.tile([1, 1], F32, tag="stot", name=f"stot{b}")
                    nc.vector.tensor_reduce(
                        stot[:], sums[:], axis=AxisListType.X, op=AluOpType.add
                    )
                    rc = spool.tile([1, 1], F32, tag="rc", name=f"rc{b}")
                    nc.vector.reciprocal(rc[:], stot[:])
                    oo = spool.tile([1, L], F32, tag="oo", name=f"oo{b}")
                    nc.vector.tensor_scalar_mul(oo[:], ex[:], rc[:, 0:1])
                    nc.sync.dma_start(out[b : b + 1, :], oo[:])

    nc.compile()
    return nc


_cached_nc = None


def _prep_in_maps(hidden, encoder_outputs, W, b, v):
    hidden = np.ascontiguousarray(hidden, dtype=np.float32)
    W = np.ascontiguousarray(W, dtype=np.float32)
    b = np.ascontiguousarray(b, dtype=np.float32)
    v = np.ascontiguousarray(v, dtype=np.float32)
    # (L, B, D) -> (B, D, L) so per-batch d-major tiles are contiguous
    encT = np.ascontiguousarray(
        np.asarray(encoder_outputs, dtype=np.float32).transpose(1, 2, 0)
    )
    wet = np.ascontiguousarray(W[:, H:].T)    # (576, 512)
    wht = np.ascontiguousarray(W[:, :H].T)    # (512, 512)
    bcol = np.ascontiguousarray(b.reshape(NHT, 128).T)  # (128, 4)
    vcol = np.ascontiguousarray(v.reshape(NHT, 128).T)  # (128, 4)
    in_maps = []
    for c in range(N_CORES):
        sl = slice(c * BL, (c + 1) * BL)
        in_maps.append(
            {
                "enc": encT[sl],
                "hid": np.ascontiguousarray(hidden[sl].T),
                "wet": wet,
                "wht": wht,
                "bcol": bcol,
                "vcol": vcol,
            }
        )
    return in_maps


def kernel(hidden, encoder_outputs, W, b, v):
    global _cached_nc
    if _cached_nc is None:
        _cached_nc = build(reps=1)
    in_maps = _prep_in_maps(hidden, encoder_outputs, W, b, v)
    res = bass_utils.run_bass_kernel_spmd(
        _cached_nc, in_maps, core_ids=list(range(N_CORES))
    )
    outs = np.concatenate([res.results[c]["out"] for c in range(N_CORES)], axis=0)
    return outs[:, None, :].astype(np.float32)


# revision 4
# speedup vs baseline: 1.1052x; 1.1052x over previous
"""Trainium2 Bass kernel for nn_Attn (Bahdanau-style attention scores).

Computation (per batch b of B=128):
    energy = tanh(enc[b] @ We.T + (hidden @ Wh.T)[b] + bias)   # (L, H)
    scores = energy @ v                                        # (L,)
    out[b] = softmax(scores)                                   # (1, L)

Sharding: batch data-parallel over 8 NeuronCores (16 batches/core);
weights replicated. Per core the dominant matmul is computed in the
[h, l] orientation so the PE tensor engine contracts over d (=576):

    part_e[h, l] = sum_d WeT[d, h] * encT[d, l]      (lhsT=WeT, rhs=encT)

which lets the (hidden@Wh.T + bias) term fuse into the tanh as a
per-partition activation bias, and the v-contraction run as a second
PE matmul (lhsT = v column, contracting over h on partitions).
Matmuls run as float32r (full fp32 data, reduced-precision multiply,
1 col/cycle on TRN2 vs 4 for exact fp32).

The contraction dim is zero-padded host-side from 576 to 640 so every
k-tile is a full 128 partitions: K=64 matmuls (and their successors)
measure ~2x slower on HW than K=128 ones, costing far more than the 11%
extra DMA.

Host side: encoder_outputs (L, B, D) is transposed once to (B, D, L) so
each per-batch d-major tile DMA is contiguous.

Scores are assembled batch-major ([16, L] via tiny SBUF->SBUF row DMAs)
so softmax runs once over all local batches at the end instead of as 16
serial per-batch chains on the ACT/DVE engines.
"""

import numpy as np

import concourse.bacc as bacc
import concourse.bass as bass
import concourse.mybir as mybir
import concourse.tile as tile
from concourse import bass_utils
from concourse.mybir import ActivationFunctionType as AF
from concourse.mybir import AluOpType, AxisListType

N_CORES = 8
B, L, H = 128, 1024, 512
ONEHOT = 64
DE = H + ONEHOT          # 576, true contraction dim of the big matmul
DP = 640                 # padded contraction dim (5 full 128-tiles)
BL = B // N_CORES        # 16 batches per core
F32 = mybir.dt.float32
F32R = mybir.dt.float32r

NKT = DP // 128                          # 5 d-tiles, all full
NHT = H // 128                           # 4 h-tiles
NLH = L // 512                           # 2 l-halves (N=512 per matmul)


def build(reps: int = 1):
    """Build + trace the per-core Bass program. Returns the compiled nc."""
    nc = bacc.Bacc(
        "TRN2", target_bir_lowering=False, debug=False, num_devices=N_CORES
    )
    enc = nc.dram_tensor("enc", [BL, DP, L], F32R, kind="ExternalInput").ap()
    hid = nc.dram_tensor("hid", [H, BL], F32, kind="ExternalInput").ap()
    wet = nc.dram_tensor("wet", [DP, H], F32R, kind="ExternalInput").ap()
    wht = nc.dram_tensor("wht", [H, H], F32, kind="ExternalInput").ap()
    bcol = nc.dram_tensor("bcol", [128, NHT], F32, kind="ExternalInput").ap()
    vcol = nc.dram_tensor("vcol", [128, NHT], F32R, kind="ExternalInput").ap()
    out = nc.dram_tensor("out", [BL, L], F32, kind="ExternalOutput").ap()

    with tile.TileContext(nc) as tc:
        with (
            tc.tile_pool(name="const", bufs=1) as cpool,
            tc.tile_pool(name="encp", bufs=3) as epool,
            tc.tile_pool(name="energy", bufs=3) as gpool,
            tc.tile_pool(name="cb", bufs=2) as cbpool,
            tc.tile_pool(name="soft", bufs=1) as spool,
            tc.tile_pool(name="stage", bufs=4) as stpool,
            tc.tile_pool(name="ps1", bufs=4, space="PSUM") as ps1,
            tc.tile_pool(name="ps2", bufs=4, space="PSUM") as ps2,
        ):
            # ---- replicated constants (gpsimd queue: don't serialize
            # behind the big enc prefetches on the sync queue) ----
            wet_sb = []
            for kt in range(NKT):
                t = cpool.tile([128, H], F32R, tag=f"wet{kt}", name=f"wet{kt}")
                nc.gpsimd.dma_start(t[:], wet[kt * 128 : (kt + 1) * 128, :])
                wet_sb.append(t)
            wht_sb = []
            for kt in range(4):
                t = cpool.tile([128, H], F32, tag=f"wht{kt}", name=f"wht{kt}")
                nc.gpsimd.dma_start(t[:], wht[kt * 128 : (kt + 1) * 128, :])
                wht_sb.append(t)
            hid_sb = []
            for kt in range(4):
                t = cpool.tile([128, BL], F32, tag=f"hid{kt}", name=f"hid{kt}")
                nc.gpsimd.dma_start(t[:], hid[kt * 128 : (kt + 1) * 128, :])
                hid_sb.append(t)
            bcol_sb = cpool.tile([128, NHT], F32, tag="bcol", name="bcol_sb")
            nc.gpsimd.dma_start(bcol_sb[:], bcol[:, :])
            vcol_sb = cpool.tile([128, NHT], F32R, tag="vcol", name="vcol_sb")
            nc.gpsimd.dma_start(vcol_sb[:], vcol[:, :])

            for _rep in range(reps):
                # ---- c[h, b] = (hidden @ Wh.T).T + bias, per-partition h ----
                cb_sb = []
                for ht in range(4):
                    pc = ps1.tile([128, 512], F32, tag="ps1", name=f"pc{ht}")
                    for kt in range(4):
                        nc.tensor.matmul(
                            pc[:, :BL],
                            lhsT=wht_sb[kt][:, ht * 128 : (ht + 1) * 128],
                            rhs=hid_sb[kt][:],
                            start=(kt == 0),
                            stop=(kt == 3),
                        )
                    cbt = cbpool.tile([128, BL], F32, tag=f"cb{ht}", name=f"cb{ht}")
                    nc.vector.tensor_scalar_add(
                        cbt[:], pc[:, :BL], bcol_sb[:, ht : ht + 1]
                    )
                    cb_sb.append(cbt)

                scores_sb = spool.tile([BL, L], F32, tag="scores", name="scores_sb")

                # ---- main loop over local batches ----
                for b in range(BL):
                    et = []
                    for kt in range(NKT):
                        t = epool.tile([128, L], F32R, tag=f"enc{kt}", name=f"enc{kt}_{b}")
                        nc.sync.dma_start(
                            t[:], enc[b, kt * 128 : (kt + 1) * 128, :]
                        )
                        et.append(t)

                    for lh in range(NLH):
                        ps_s = ps2.tile([1, 512], F32, tag="ps2", name=f"ps_s{b}_{lh}")
                        for ht in range(4):
                            pe_t = ps1.tile(
                                [128, 512], F32, tag="ps1", name=f"pe{b}_{lh}_{ht}"
                            )
                            for kt in range(NKT):
                                nc.tensor.matmul(
                                    pe_t[:],
                                    lhsT=wet_sb[kt][:, ht * 128 : (ht + 1) * 128],
                                    rhs=et[kt][:, lh * 512 : (lh + 1) * 512],
                                    start=(kt == 0),
                                    stop=(kt == NKT - 1),
                                )
                            en_t = gpool.tile(
                                [128, 512], F32R, tag="en", name=f"en{b}_{lh}_{ht}"
                            )
                            nc.scalar.activation(
                                en_t[:], pe_t[:], AF.Tanh,
                                bias=cb_sb[ht][:, b : b + 1],
                            )
                            nc.tensor.matmul(
                                ps_s[:],
                                lhsT=vcol_sb[:, ht : ht + 1],
                                rhs=en_t[:],
                                start=(ht == 0),
                                stop=(ht == 3),
                            )
                        # stage psum scores out and park them batch-major
                        st = stpool.tile([1, 512], F32, tag="st", name=f"st{b}_{lh}")
                        nc.vector.tensor_copy(st[:], ps_s[:])
                        nc.sync.dma_start(
                            scores_sb[b : b + 1, lh * 512 : (lh + 1) * 512], st[:]
                        )

                # ---- one softmax over all local batches ----
                mx = spool.tile([BL, 1], F32, tag="mx", name="mx")
                nc.vector.tensor_reduce(
                    mx[:], scores_sb[:], axis=AxisListType.X, op=AluOpType.max,
                    negate=True,
                )
                ex = spool.tile([BL, L], F32, tag="ex", name="ex")
                sm = spool.tile([BL, 1], F32, tag="sm", name="sm")
                nc.scalar.activation(
                    ex[:], scores_sb[:], AF.Exp, bias=mx[:, 0:1],
                    accum_out=sm[:],
                )
                rc = spool.tile([BL, 1], F32, tag="rc", name="rc")
                nc.vector.reciprocal(rc[:], sm[:])
                oo = spool.tile([BL, L], F32, tag="oo", name="oo")
                nc.vector.tensor_scalar_mul(oo[:], ex[:], rc[:, 0:1])
                nc.sync.dma_start(out[:, :], oo[:])

    nc.compile()
    return nc


_cached_nc = None


def _prep_in_maps(hidden, encoder_outputs, W, b, v):
    hidden = np.ascontiguousarray(hidden, dtype=np.float32)
    W = np.ascontiguousarray(W, dtype=np.float32)
    b = np.ascontiguousarray(b, dtype=np.float32)
    v = np.ascontiguousarray(v, dtype=np.float32)
    # (L, B, D) -> (B, D, L), zero-padded to DP on the d axis
    e = np.asarray(encoder_outputs, dtype=np.float32)
    encT = np.zeros((B, DP, L), dtype=np.float32)
    encT[:, :DE, :] = e.transpose(1, 2, 0)
    wet = np.zeros((DP, H), dtype=np.float32)
    wet[:DE] = W[:, H:].T                               # We.T (padded)
    wht = np.ascontiguousarray(W[:, :H].T)              # (512, 512)
    bcol = np.ascontiguousarray(b.reshape(NHT, 128).T)  # (128, 4)
    vcol = np.ascontiguousarray(v.reshape(NHT, 128).T)  # (128, 4)
    in_maps = []
    for c in range(N_CORES):
        sl = slice(c * BL, (c + 1) * BL)
        in_maps.append(
            {
                "enc": encT[sl],
                "hid": np.ascontiguousarray(hidden[sl].T),
                "wet": wet,
                "wht": wht,
                "bcol": bcol,
                "vcol": vcol,
            }
        )
    return in_maps


def kernel(hidden, encoder_outputs, W, b, v):
    global _cached_nc
    if _cached_nc is None:
        _cached_nc = build(reps=1)
    in_maps = _prep_in_maps(hidden, encoder_outputs, W, b, v)
    res = bass_utils.run_bass_kernel_spmd(
        _cached_nc, in_maps, core_ids=list(range(N_CORES))
    )
    outs = np.concatenate([res.results[c]["out"] for c in range(N_CORES)], axis=0)
    return outs[:, None, :].astype(np.float32)


# revision 5
# speedup vs baseline: 1.1652x; 1.0542x over previous
"""Trainium2 Bass kernel for nn_Attn (Bahdanau-style attention scores).

Computation (per batch b of B=128):
    energy = tanh(enc[b] @ We.T + (hidden @ Wh.T)[b] + bias)   # (L, H)
    scores = energy @ v                                        # (L,)
    out[b] = softmax(scores)                                   # (1, L)

Sharding: batch data-parallel over 8 NeuronCores (16 batches/core);
weights replicated. Per core the dominant matmul is computed in the
[h, l] orientation so the PE tensor engine contracts over d (=576):

    part_e[h, l] = sum_d WeT[d, h] * encT[d, l]      (lhsT=WeT, rhs=encT)

which lets the (hidden@Wh.T + bias) term fuse into the tanh as a
per-partition activation bias, and the v-contraction run as a second
PE matmul (lhsT = v column, contracting over h on partitions).
Matmuls run as float32r (full fp32 data, reduced-precision multiply,
1 col/cycle on TRN2 vs 4 for exact fp32).

The contraction dim is zero-padded host-side from 576 to 640 so every
k-tile is a full 128 partitions: K=64 matmuls (and their successors)
measure ~2x slower on HW than K=128 ones, costing far more than the 11%
extra DMA.

Host side: encoder_outputs (L, B, D) is transposed once to (B, D, L) so
each per-batch d-major tile DMA is contiguous.

Scores are assembled batch-major ([16, L] via tiny SBUF->SBUF row DMAs)
so softmax runs once over all local batches at the end instead of as 16
serial per-batch chains on the ACT/DVE engines.
"""

import numpy as np

import concourse.bacc as bacc
import concourse.bass as bass
import concourse.mybir as mybir
import concourse.tile as tile
from concourse import bass_utils
from concourse.mybir import ActivationFunctionType as AF
from concourse.mybir import AluOpType, AxisListType

N_CORES = 8
B, L, H = 128, 1024, 512
ONEHOT = 64
DE = H + ONEHOT          # 576, true contraction dim of the big matmul
DP = 640                 # padded contraction dim (5 full 128-tiles)
BL = B // N_CORES        # 16 batches per core
F32 = mybir.dt.float32
F32R = mybir.dt.float32r

NKT = DP // 128                          # 5 d-tiles, all full
NHT = H // 128                           # 4 h-tiles
NLH = L // 512                           # 2 l-halves (N=512 per matmul)


def build(reps: int = 1):
    """Build + trace the per-core Bass program. Returns the compiled nc."""
    nc = bacc.Bacc(
        "TRN2", target_bir_lowering=False, debug=False, num_devices=N_CORES
    )
    enc = nc.dram_tensor("enc", [BL, DP, L], F32R, kind="ExternalInput").ap()
    hid = nc.dram_tensor("hid", [H, BL], F32, kind="ExternalInput").ap()
    wet = nc.dram_tensor("wet", [DP, H], F32R, kind="ExternalInput").ap()
    wht = nc.dram_tensor("wht", [H, H], F32, kind="ExternalInput").ap()
    bcol = nc.dram_tensor("bcol", [128, NHT], F32, kind="ExternalInput").ap()
    vcol = nc.dram_tensor("vcol", [128, NHT], F32R, kind="ExternalInput").ap()
    out = nc.dram_tensor("out", [BL, L], F32, kind="ExternalOutput").ap()

    with tile.TileContext(nc) as tc:
        with (
            tc.tile_pool(name="const", bufs=1) as cpool,
            tc.tile_pool(name="encp", bufs=3) as epool,
            tc.tile_pool(name="energy", bufs=3) as gpool,
            tc.tile_pool(name="cb", bufs=2) as cbpool,
            tc.tile_pool(name="soft", bufs=1) as spool,
            tc.tile_pool(name="stage", bufs=4) as stpool,
            tc.tile_pool(name="ps1", bufs=4, space="PSUM") as ps1,
            tc.tile_pool(name="ps2", bufs=4, space="PSUM") as ps2,
        ):
            # ---- replicated constants (gpsimd queue: don't serialize
            # behind the big enc prefetches on the sync queue) ----
            wet_sb = []
            for kt in range(NKT):
                t = cpool.tile([128, H], F32R, tag=f"wet{kt}", name=f"wet{kt}")
                nc.sync.dma_start(t[:], wet[kt * 128 : (kt + 1) * 128, :])
                wet_sb.append(t)
            wht_sb = []
            for kt in range(4):
                t = cpool.tile([128, H], F32, tag=f"wht{kt}", name=f"wht{kt}")
                nc.sync.dma_start(t[:], wht[kt * 128 : (kt + 1) * 128, :])
                wht_sb.append(t)
            hid_sb = []
            for kt in range(4):
                t = cpool.tile([128, BL], F32, tag=f"hid{kt}", name=f"hid{kt}")
                nc.sync.dma_start(t[:], hid[kt * 128 : (kt + 1) * 128, :])
                hid_sb.append(t)
            bcol_sb = cpool.tile([128, NHT], F32, tag="bcol", name="bcol_sb")
            nc.sync.dma_start(bcol_sb[:], bcol[:, :])
            vcol_sb = cpool.tile([128, NHT], F32R, tag="vcol", name="vcol_sb")
            nc.sync.dma_start(vcol_sb[:], vcol[:, :])

            for _rep in range(reps):
                # ---- c[h, b] = (hidden @ Wh.T).T + bias, per-partition h ----
                cb_sb = []
                for ht in range(4):
                    pc = ps1.tile([128, 512], F32, tag="ps1", name=f"pc{ht}")
                    for kt in range(4):
                        nc.tensor.matmul(
                            pc[:, :BL],
                            lhsT=wht_sb[kt][:, ht * 128 : (ht + 1) * 128],
                            rhs=hid_sb[kt][:],
                            start=(kt == 0),
                            stop=(kt == 3),
                        )
                    cbt = cbpool.tile([128, BL], F32, tag=f"cb{ht}", name=f"cb{ht}")
                    nc.vector.tensor_scalar_add(
                        cbt[:], pc[:, :BL], bcol_sb[:, ht : ht + 1]
                    )
                    cb_sb.append(cbt)

                scores_sb = spool.tile([BL, L], F32, tag="scores", name="scores_sb")

                # ---- main loop over local batches ----
                for b in range(BL):
                    et = []
                    for kt in range(NKT):
                        t = epool.tile([128, L], F32R, tag=f"enc{kt}", name=f"enc{kt}_{b}")
                        nc.sync.dma_start(
                            t[:], enc[b, kt * 128 : (kt + 1) * 128, :]
                        )
                        et.append(t)

                    for lh in range(NLH):
                        ps_s = ps2.tile([1, 512], F32, tag="ps2", name=f"ps_s{b}_{lh}")
                        for ht in range(4):
                            pe_t = ps1.tile(
                                [128, 512], F32, tag="ps1", name=f"pe{b}_{lh}_{ht}"
                            )
                            for kt in range(NKT):
                                nc.tensor.matmul(
                                    pe_t[:],
                                    lhsT=wet_sb[kt][:, ht * 128 : (ht + 1) * 128],
                                    rhs=et[kt][:, lh * 512 : (lh + 1) * 512],
                                    start=(kt == 0),
                                    stop=(kt == NKT - 1),
                                )
                            en_t = gpool.tile(
                                [128, 512], F32R, tag="en", name=f"en{b}_{lh}_{ht}"
                            )
                            nc.scalar.activation(
                                en_t[:], pe_t[:], AF.Tanh,
                                bias=cb_sb[ht][:, b : b + 1],
                            )
                            nc.tensor.matmul(
                                ps_s[:],
                                lhsT=vcol_sb[:, ht : ht + 1],
                                rhs=en_t[:],
                                start=(ht == 0),
                                stop=(ht == 3),
                            )
                        # stage psum scores out and park them batch-major
                        st = stpool.tile([1, 512], F32, tag="st", name=f"st{b}_{lh}")
                        nc.vector.tensor_copy(st[:], ps_s[:])
                        nc.sync.dma_start(
                            scores_sb[b : b + 1, lh * 512 : (lh + 1) * 512], st[:]
                        )

                # ---- one softmax over all local batches ----
                mx = spool.tile([BL, 1], F32, tag="mx", name="mx")
                nc.vector.tensor_reduce(
                    mx[:], scores_sb[:], axis=AxisListType.X, op=AluOpType.max,
                    negate=True,
                )
                ex = spool.tile([BL, L], F32, tag="ex", name="ex")
                sm = spool.tile([BL, 1], F32, tag="sm", name="sm")
                nc.scalar.activation(
                    ex[:], scores_sb[:], AF.Exp, bias=mx[:, 0:1],
                    accum_out=sm[:],
                )
                rc = spool.tile([BL, 1], F32, tag="rc", name="rc")
                nc.vector.reciprocal(rc[:], sm[:])
                oo = spool.tile([BL, L], F32, tag="oo", name="oo")
                nc.vector.tensor_scalar_mul(oo[:], ex[:], rc[:, 0:1])
                nc.sync.dma_start(out[:, :], oo[:])

    nc.compile()
    return nc


_cached_nc = None


def _prep_in_maps(hidden, encoder_outputs, W, b, v):
    hidden = np.ascontiguousarray(hidden, dtype=np.float32)
    W = np.ascontiguousarray(W, dtype=np.float32)
    b = np.ascontiguousarray(b, dtype=np.float32)
    v = np.ascontiguousarray(v, dtype=np.float32)
    # (L, B, D) -> (B, D, L), zero-padded to DP on the d axis
    e = np.asarray(encoder_outputs, dtype=np.float32)
    encT = np.zeros((B, DP, L), dtype=np.float32)
    encT[:, :DE, :] = e.transpose(1, 2, 0)
    wet = np.zeros((DP, H), dtype=np.float32)
    wet[:DE] = W[:, H:].T                               # We.T (padded)
    wht = np.ascontiguousarray(W[:, :H].T)              # (512, 512)
    bcol = np.ascontiguousarray(b.reshape(NHT, 128).T)  # (128, 4)
    vcol = np.ascontiguousarray(v.reshape(NHT, 128).T)  # (128, 4)
    in_maps = []
    for c in range(N_CORES):
        sl = slice(c * BL, (c + 1) * BL)
        in_maps.append(
            {
                "enc": encT[sl],
                "hid": np.ascontiguousarray(hidden[sl].T),
                "wet": wet,
                "wht": wht,
                "bcol": bcol,
                "vcol": vcol,
            }
        )
    return in_maps


def kernel(hidden, encoder_outputs, W, b, v):
    global _cached_nc
    if _cached_nc is None:
        _cached_nc = build(reps=1)
    in_maps = _prep_in_maps(hidden, encoder_outputs, W, b, v)
    res = bass_utils.run_bass_kernel_spmd(
        _cached_nc, in_maps, core_ids=list(range(N_CORES))
    )
    outs = np.concatenate([res.results[c]["out"] for c in range(N_CORES)], axis=0)
    return outs[:, None, :].astype(np.float32)
